# revision 23
# baseline (speedup 1.0000x reference)
"""Trainium2 Bass kernel for CalibrationFreeFP8Linear.

Computes: quantize x and w to fp8-e4m3 with EMA-updated dynamic absmax
scales, fp8 matmul (fp32 accumulate), dequantize, cast to bf16.

Sharding: data-parallel over the 16384 (B*S) rows of x across 8 cores;
weight replicated. The x absmax needs a global max -> AllReduce(max).

Host side pre-transposes both operands to K-major layout ([K, M] / [K, N])
so the tensor engine (which contracts over the partition axis for both
operands) gets contiguous DMA loads with no on-device transpose.

Structure (per core):
  1. x loaded first (resident bf16) -- x gates the absmax collective,
     the longest dependency chain. w loaded second (also resident;
     both bf16 + both fp8 copies = ~205KB/partition, fits in the
     207.9KB SBUF budget).
  2. absmax on DVE while loads stream.
  3. 512B AllReduce(max) for x's global absmax; w's absmax is local.
  4. quantize (DVE tensor_scalar 4x for x; w split DVE/ACT), fp8
     DoubleRow matmuls m->kk->n, dequant epilogue on ACT, DMA out.
"""

import numpy as np
import ml_dtypes

import concourse.bass as bass
import concourse.mybir as mybir
import concourse.tile as tile
from concourse import bacc, bass_isa
from concourse.bass import ts
from concourse.bass_utils import run_bass_kernel_spmd

FP8_MAX = 448.0
EMA = 0.9
N_CORES = 8
P = 128

# Full problem shapes (hardcoded; kernel.py must be self-contained).
B, S, K, N = 4, 4096, 2048, 2048
M_PER_CORE = (B * S) // N_CORES  # 2048

# absmax implementation: "ts_accum" (tensor_scalar 4x with max/min
# accumulators), "act_abs" (ACT abs + DVE 1x reduce), or
# "reduce" (plain 1x tensor_reduce).
ABSMAX_MODE = "reduce"


def build_nc(M, K, N, n_cores=N_CORES, dma_k=1, absmax_mode=None):
    """Build the SPMD Bass program for one core's [M, K] @ [K, N]^T shard.

    DRAM inputs (per core): xt [K, M] bf16, wt [K, N] bf16 (both K-major),
    in_s [1] f32, w_s [1] f32. Output: out [M, N] bf16.
    """
    if absmax_mode is None:
        absmax_mode = ABSMAX_MODE
    dt = mybir.dt
    KT = K // P            # k-subtiles
    MT = M // P            # m-tiles
    N_TILE = min(512, N)
    NT = N // N_TILE
    assert K % P == 0 and M % P == 0 and N % N_TILE == 0
    assert KT % 2 == 0, "DoubleRow needs an even number of k-subtiles"
    assert KT % dma_k == 0
    n_dma = KT // dma_k
    CHUNK_F = dma_k * max(M, N)

    nc = bacc.Bacc(
        "TRN2",
        target_bir_lowering=False,
        debug=False,
        num_devices=n_cores,
    )

    xt = nc.dram_tensor("xt", [K, M], dt.bfloat16, kind="ExternalInput").ap()
    wt = nc.dram_tensor("wt", [K, N], dt.bfloat16, kind="ExternalInput").ap()
    in_s = nc.dram_tensor("in_s", [1], dt.float32, kind="ExternalInput").ap()
    w_s = nc.dram_tensor("w_s", [1], dt.float32, kind="ExternalInput").ap()
    out = nc.dram_tensor("out", [M, N], dt.bfloat16, kind="ExternalOutput").ap()

    # K-major views of the DRAM tensors: k = ko*P + p
    xt_v = xt.rearrange("(ko p) m -> p ko m", p=P)
    wt_v = wt.rearrange("(ko p) n -> p ko n", p=P)
    out_v = out.rearrange("(mo p) n -> p mo n", p=P)

    rg = [list(range(n_cores))]
    MX = mybir.AluOpType.max
    MN = mybir.AluOpType.min

    with tile.TileContext(nc) as tc:
        with (
            tc.tile_pool(name="stats", bufs=1) as stats,
            tc.tile_pool(name="dram", bufs=1, space="DRAM") as dram,
            tc.tile_pool(name="xb_pool", bufs=1) as xb_pool,
            tc.tile_pool(name="wb_pool", bufs=1) as wb_pool,
            tc.tile_pool(name="wf_pool", bufs=1) as wf_pool,
            tc.tile_pool(name="xf_pool", bufs=1) as xf_pool,
            tc.tile_pool(name="psum", bufs=max(1, 8 // NT), space="PSUM") as psum,
            tc.tile_pool(name="outp", bufs=4) as outp,
        ):
            # ---- absmax machinery (per DMA chunk, on DVE)
            if absmax_mode != "reduce":
                sq_scratch = stats.tile([P, CHUNK_F], dt.bfloat16)
            rc_max = stats.tile([P, 2 * n_dma], dt.float32)
            rc_min = stats.tile([P, 2 * n_dma], dt.float32)
            _ci = [0]

            def absmax_chunk(chunk2d, col, first, last):
                """Feed one [P, F] bf16 chunk; when `last`, write the
                per-partition absmax of the whole tensor into f32 col."""
                j = _ci[0]
                _ci[0] += 1
                F = chunk2d.shape[-1]
                if absmax_mode == "ts_accum":
                    # 4x DVE pass each for running max and min columns
                    nc.vector.tensor_scalar(
                        sq_scratch[:, :F], chunk2d, 1.0, None,
                        mybir.AluOpType.mult, MX, accum_out=rc_max[:, j : j + 1],
                    )
                    nc.vector.tensor_scalar(
                        sq_scratch[:, :F], chunk2d, 1.0, None,
                        mybir.AluOpType.mult, MN, accum_out=rc_min[:, j : j + 1],
                    )
                elif absmax_mode == "act_abs":
                    nc.scalar.activation(
                        sq_scratch[:, :F], chunk2d, mybir.ActivationFunctionType.Abs
                    )
                    nc.vector.tensor_reduce(
                        rc_max[:, j : j + 1], sq_scratch[:, :F],
                        axis=mybir.AxisListType.X, op=MX,
                    )
                else:  # plain 1x reduce
                    nc.vector.tensor_reduce(
                        rc_max[:, j : j + 1], chunk2d,
                        axis=mybir.AxisListType.X, op=MX,
                        apply_absolute_value=True,
                    )
                if last:
                    base = j + 1 - n_dma
                    sl = slice(base, base + n_dma)
                    nc.vector.tensor_reduce(
                        col, rc_max[:, sl], axis=mybir.AxisListType.X, op=MX
                    )
                    if absmax_mode == "ts_accum":
                        neg = stats.tile([P, 1], dt.float32, name=f"neg_{j}")
                        nc.vector.tensor_reduce(
                            neg, rc_min[:, sl], axis=mybir.AxisListType.X, op=MN
                        )
                        nc.vector.tensor_scalar_mul(neg, neg, -1.0)
                        nc.vector.tensor_tensor(col, col, neg, MX)

            # EMA scale inputs: tiny loads issued first on the DMA queue
            prev_s = stats.tile([1, 2], dt.float32)
            nc.sync.dma_start(prev_s[:, 0:1], in_s.rearrange("(o p) -> p o", p=1))
            nc.sync.dma_start(prev_s[:, 1:2], w_s.rearrange("(o p) -> p o", p=1))
            prev_b = stats.tile([P, 2], dt.float32)
            nc.gpsimd.partition_broadcast(prev_b, prev_s, channels=P)

            # ---- x loads (resident) + absmax
            xb = xb_pool.tile([P, KT, M], dt.bfloat16)
            amax_x = stats.tile([P, 1], dt.float32)
            for j in range(n_dma):
                nc.sync.dma_start(xb[:, ts(j, dma_k)], xt_v[:, ts(j, dma_k)])
                chunk2d = xb[:, ts(j, dma_k)].rearrange("p a b -> p (a b)")
                absmax_chunk(chunk2d, amax_x, j == 0, j == n_dma - 1)

            # ---- w loads (resident) + absmax
            wb = wb_pool.tile([P, KT, N], dt.bfloat16)
            amax_w = stats.tile([P, 1], dt.float32)
            for j in range(n_dma):
                nc.sync.dma_start(wb[:, ts(j, dma_k)], wt_v[:, ts(j, dma_k)])
                chunk2d = wb[:, ts(j, dma_k)].rearrange("p a b -> p (a b)")
                absmax_chunk(chunk2d, amax_w, j == 0, j == n_dma - 1)

            amax_x_b = stats.tile([P, 1], dt.float32)
            nc.gpsimd.partition_all_reduce(
                amax_x_b, amax_x, channels=P, reduce_op=bass_isa.ReduceOp.max
            )

            # ---- global x absmax across cores. AllGather of each core's
            # (uniform) 512B absmax column + a local max over the gathered
            # values -- AllGather has a much lower latency floor than
            # AllReduce (one wire pass, no CCE reduce).
            cc_in = dram.tile([P], dt.float32)
            cc_in_v = cc_in.rearrange("(o p) -> p o", p=P)
            nc.sync.dma_start(cc_in_v, amax_x_b)
            if n_cores > 1:
                cc_out = dram.tile([n_cores * P], dt.float32, addr_space="Shared")
                nc.gpsimd.collective_compute(
                    "AllGather",
                    mybir.AluOpType.bypass,
                    replica_groups=rg,
                    ins=[cc_in.opt()],
                    outs=[cc_out.opt()],
                )
                gath = stats.tile([1, n_cores * P], dt.float32)
                nc.sync.dma_start(gath, cc_out.rearrange("(o f) -> o f", o=1))
            else:
                gath = None

            amax_w_b = stats.tile([P, 1], dt.float32)
            nc.gpsimd.partition_all_reduce(
                amax_w_b, amax_w, channels=P, reduce_op=bass_isa.ReduceOp.max
            )

            def ema_scale(amax_col, prev_col, name):
                t = stats.tile([P, 1], dt.float32, name=f"t_{name}")
                nc.vector.tensor_scalar_add(t, amax_col, 1e-12)
                nc.vector.reciprocal(t, t)
                nc.vector.tensor_scalar_mul(t, t, FP8_MAX)
                nc.vector.tensor_scalar(
                    t, t, 1e-6, 1e6, mybir.AluOpType.max, mybir.AluOpType.min
                )
                s = stats.tile([P, 1], dt.float32, name=f"s_{name}")
                nc.vector.tensor_scalar_mul(s, t, 1.0 - EMA)
                t2 = stats.tile([P, 1], dt.float32, name=f"t2_{name}")
                nc.vector.tensor_scalar_mul(t2, prev_col, EMA)
                nc.vector.tensor_add(s, s, t2)
                return s

            # w-side chain first: nothing here depends on the collective,
            # so DVE stays busy while the AllReduce is in flight.
            s_w = ema_scale(amax_w_b, prev_b[:, 1:2], "w")
            wf = wf_pool.tile([P, KT, N], dt.float8e4)
            n_act = max(1, (3 * n_dma) // 8)  # tail chunks quantized on ACT
            for j in range(n_dma - n_act):
                nc.vector.tensor_scalar_mul(
                    wf[:, ts(j, dma_k)], wb[:, ts(j, dma_k)], s_w
                )
            for j in range(n_dma - n_act, n_dma):
                for t in range(dma_k):
                    nc.scalar.mul(wf[:, j * dma_k + t], wb[:, j * dma_k + t], mul=s_w)

            # x-side chain: gated by the collective result. The local
            # reduction of the gathered absmax values is emitted here, after
            # all w-side DVE work, so the collective wait can't stall it.
            if gath is not None:
                red = stats.tile([1, 1], dt.float32)
                nc.vector.tensor_reduce(
                    red, gath, axis=mybir.AxisListType.X, op=MX
                )
                amax_x_g = stats.tile([P, 1], dt.float32)
                nc.gpsimd.partition_broadcast(amax_x_g, red, channels=P)
            else:
                amax_x_g = amax_x_b
            s_x = ema_scale(amax_x_g, prev_b[:, 0:1], "x")
            xf = xf_pool.tile([P, KT, M], dt.float8e4)
            for j in range(n_dma - n_act):
                nc.vector.tensor_scalar_mul(
                    xf[:, ts(j, dma_k)], xb[:, ts(j, dma_k)], s_x
                )
            for j in range(n_dma - n_act, n_dma):
                for t in range(dma_k):
                    nc.scalar.mul(xf[:, j * dma_k + t], xb[:, j * dma_k + t], mul=s_x)

            # inv = 1 / (s_x * s_w) for the output dequant (emitted after the
            # quantize loops so it doesn't delay the first xf chunk)
            inv = stats.tile([P, 1], dt.float32)
            nc.vector.tensor_mul(inv, s_x, s_w)
            nc.vector.reciprocal(inv, inv)

            # ---- fp8 DoubleRow matmul + dequant epilogue
            for m in range(MT):
                pts = [
                    psum.tile([P, N_TILE], dt.float32, name=f"pt{n}") for n in range(NT)
                ]
                for kk in range(KT // 2):
                    for n in range(NT):
                        nc.tensor.matmul(
                            pts[n],
                            xf[:, 2 * kk : 2 * kk + 2, ts(m, P)],
                            wf[:, 2 * kk : 2 * kk + 2, ts(n, N_TILE)],
                            start=(kk == 0),
                            stop=(kk == KT // 2 - 1),
                            perf_mode=mybir.MatmulPerfMode.DoubleRow,
                        )
                for n in range(NT):
                    out_mn = outp.tile([P, N_TILE], dt.bfloat16, name="out_mn")
                    nc.scalar.mul(out_mn, pts[n], mul=inv)
                    nc.sync.dma_start(out_v[:, m, ts(n, N_TILE)], out_mn)

    nc.compile()
    return nc


_NC_CACHE = {}


def _get_nc(M, K, N, n_cores=N_CORES):
    key = (M, K, N, n_cores)
    if key not in _NC_CACHE:
        _NC_CACHE[key] = build_nc(M, K, N, n_cores)
    return _NC_CACHE[key]


def run_sharded(x2d, weight, input_scale, weight_scale, n_cores=N_CORES, trace=False):
    """x2d: [rows, K] bf16, weight: [N, K] bf16. Returns ([rows, N] bf16, result)."""
    rows, k = x2d.shape
    n = weight.shape[0]
    m_per = rows // n_cores
    nc = _get_nc(m_per, k, n, n_cores)

    wt = np.ascontiguousarray(weight.T)  # [K, N]
    in_s = np.asarray(input_scale, dtype=np.float32).reshape(1)
    w_s = np.asarray(weight_scale, dtype=np.float32).reshape(1)
    in_maps = []
    for i in range(n_cores):
        xt_i = np.ascontiguousarray(x2d[i * m_per : (i + 1) * m_per].T)  # [K, M]
        in_maps.append({"xt": xt_i, "wt": wt, "in_s": in_s, "w_s": w_s})

    res = run_bass_kernel_spmd(nc, in_maps, core_ids=list(range(n_cores)), trace=trace)
    out = np.concatenate([res.results[i]["out"] for i in range(n_cores)], axis=0)
    return out, res


def kernel(x, weight, input_scale, weight_scale):
    x = np.asarray(x)
    weight = np.asarray(weight)
    b, s, k = x.shape
    x2d = np.ascontiguousarray(x.reshape(b * s, k))
    out, _ = run_sharded(x2d, weight, input_scale, weight_scale)
    return out.reshape(b, s, weight.shape[0]).astype(ml_dtypes.bfloat16)


# revision 24
# speedup vs baseline: 1.0192x; 1.0192x over previous
"""Trainium2 Bass kernel for CalibrationFreeFP8Linear.

Computes: quantize x and w to fp8-e4m3 with EMA-updated dynamic absmax
scales, fp8 matmul (fp32 accumulate), dequantize, cast to bf16.

Sharding: data-parallel over the 16384 (B*S) rows of x across 8 cores;
weight replicated. The x absmax needs a global max -> AllReduce(max).

Host side pre-transposes both operands to K-major layout ([K, M] / [K, N])
so the tensor engine (which contracts over the partition axis for both
operands) gets contiguous DMA loads with no on-device transpose.

Structure (per core):
  1. x loaded first (resident bf16) -- x gates the absmax collective,
     the longest dependency chain. w loaded second (also resident;
     both bf16 + both fp8 copies = ~205KB/partition, fits in the
     207.9KB SBUF budget).
  2. absmax on DVE while loads stream.
  3. 512B AllReduce(max) for x's global absmax; w's absmax is local.
  4. quantize (DVE tensor_scalar 4x for x; w split DVE/ACT), fp8
     DoubleRow matmuls m->kk->n, dequant epilogue on ACT, DMA out.
"""

import numpy as np
import ml_dtypes

import concourse.bass as bass
import concourse.mybir as mybir
import concourse.tile as tile
from concourse import bacc, bass_isa
from concourse.bass import ts
from concourse.bass_utils import run_bass_kernel_spmd

FP8_MAX = 448.0
EMA = 0.9
N_CORES = 8
P = 128

# Full problem shapes (hardcoded; kernel.py must be self-contained).
B, S, K, N = 4, 4096, 2048, 2048
M_PER_CORE = (B * S) // N_CORES  # 2048

# absmax implementation: "ts_accum" (tensor_scalar 4x with max/min
# accumulators), "act_abs" (ACT abs + DVE 1x reduce), or
# "reduce" (plain 1x tensor_reduce).
ABSMAX_MODE = "reduce"


def build_nc(M, K, N, n_cores=N_CORES, dma_k=1, absmax_mode=None):
    """Build the SPMD Bass program for one core's [M, K] @ [K, N]^T shard.

    DRAM inputs (per core): xt [K, M] bf16, wt [K, N] bf16 (both K-major),
    in_s [1] f32, w_s [1] f32. Output: out [M, N] bf16.
    """
    if absmax_mode is None:
        absmax_mode = ABSMAX_MODE
    dt = mybir.dt
    KT = K // P            # k-subtiles
    MT = M // P            # m-tiles
    N_TILE = min(512, N)
    NT = N // N_TILE
    assert K % P == 0 and M % P == 0 and N % N_TILE == 0
    assert KT % 2 == 0, "DoubleRow needs an even number of k-subtiles"
    assert KT % dma_k == 0
    n_dma = KT // dma_k
    CHUNK_F = dma_k * max(M, N)

    nc = bacc.Bacc(
        "TRN2",
        target_bir_lowering=False,
        debug=False,
        num_devices=n_cores,
    )

    xt = nc.dram_tensor("xt", [K, M], dt.bfloat16, kind="ExternalInput").ap()
    wt = nc.dram_tensor("wt", [K, N], dt.bfloat16, kind="ExternalInput").ap()
    in_s = nc.dram_tensor("in_s", [1], dt.float32, kind="ExternalInput").ap()
    w_s = nc.dram_tensor("w_s", [1], dt.float32, kind="ExternalInput").ap()
    out = nc.dram_tensor("out", [M, N], dt.bfloat16, kind="ExternalOutput").ap()

    # K-major views of the DRAM tensors: k = ko*P + p
    xt_v = xt.rearrange("(ko p) m -> p ko m", p=P)
    wt_v = wt.rearrange("(ko p) n -> p ko n", p=P)
    out_v = out.rearrange("(mo p) n -> p mo n", p=P)

    rg = [list(range(n_cores))]
    MX = mybir.AluOpType.max
    MN = mybir.AluOpType.min

    with tile.TileContext(nc) as tc:
        with (
            tc.tile_pool(name="stats", bufs=1) as stats,
            tc.tile_pool(name="dram", bufs=1, space="DRAM") as dram,
            tc.tile_pool(name="xb_pool", bufs=1) as xb_pool,
            tc.tile_pool(name="wb_pool", bufs=1) as wb_pool,
            tc.tile_pool(name="wf_pool", bufs=1) as wf_pool,
            tc.tile_pool(name="xf_pool", bufs=1) as xf_pool,
            tc.tile_pool(name="psum", bufs=max(1, 8 // NT), space="PSUM") as psum,
            tc.tile_pool(name="outp", bufs=4) as outp,
        ):
            # ---- absmax machinery (per DMA chunk, on DVE)
            if absmax_mode != "reduce":
                sq_scratch = stats.tile([P, CHUNK_F], dt.bfloat16)
            rc_max = stats.tile([P, 2 * n_dma], dt.float32)
            rc_min = stats.tile([P, 2 * n_dma], dt.float32)
            _ci = [0]

            def absmax_chunk(chunk2d, col, first, last):
                """Feed one [P, F] bf16 chunk; when `last`, write the
                per-partition absmax of the whole tensor into f32 col."""
                j = _ci[0]
                _ci[0] += 1
                F = chunk2d.shape[-1]
                if absmax_mode == "ts_accum":
                    # 4x DVE pass each for running max and min columns
                    nc.vector.tensor_scalar(
                        sq_scratch[:, :F], chunk2d, 1.0, None,
                        mybir.AluOpType.mult, MX, accum_out=rc_max[:, j : j + 1],
                    )
                    nc.vector.tensor_scalar(
                        sq_scratch[:, :F], chunk2d, 1.0, None,
                        mybir.AluOpType.mult, MN, accum_out=rc_min[:, j : j + 1],
                    )
                elif absmax_mode == "act_abs":
                    nc.scalar.activation(
                        sq_scratch[:, :F], chunk2d, mybir.ActivationFunctionType.Abs
                    )
                    nc.vector.tensor_reduce(
                        rc_max[:, j : j + 1], sq_scratch[:, :F],
                        axis=mybir.AxisListType.X, op=MX,
                    )
                else:  # plain 1x reduce
                    nc.vector.tensor_reduce(
                        rc_max[:, j : j + 1], chunk2d,
                        axis=mybir.AxisListType.X, op=MX,
                        apply_absolute_value=True,
                    )
                if last:
                    base = j + 1 - n_dma
                    sl = slice(base, base + n_dma)
                    nc.vector.tensor_reduce(
                        col, rc_max[:, sl], axis=mybir.AxisListType.X, op=MX
                    )
                    if absmax_mode == "ts_accum":
                        neg = stats.tile([P, 1], dt.float32, name=f"neg_{j}")
                        nc.vector.tensor_reduce(
                            neg, rc_min[:, sl], axis=mybir.AxisListType.X, op=MN
                        )
                        nc.vector.tensor_scalar_mul(neg, neg, -1.0)
                        nc.vector.tensor_tensor(col, col, neg, MX)

            # EMA scale inputs: tiny loads issued first on the DMA queue
            prev_s = stats.tile([1, 2], dt.float32)
            nc.sync.dma_start(prev_s[:, 0:1], in_s.rearrange("(o p) -> p o", p=1))
            nc.sync.dma_start(prev_s[:, 1:2], w_s.rearrange("(o p) -> p o", p=1))
            prev_b = stats.tile([P, 2], dt.float32)
            nc.gpsimd.partition_broadcast(prev_b, prev_s, channels=P)

            # ---- x loads (resident) + absmax
            xb = xb_pool.tile([P, KT, M], dt.bfloat16)
            amax_x = stats.tile([P, 1], dt.float32)
            for j in range(n_dma):
                nc.sync.dma_start(xb[:, ts(j, dma_k)], xt_v[:, ts(j, dma_k)])
                chunk2d = xb[:, ts(j, dma_k)].rearrange("p a b -> p (a b)")
                absmax_chunk(chunk2d, amax_x, j == 0, j == n_dma - 1)

            # ---- w loads (resident) + absmax
            wb = wb_pool.tile([P, KT, N], dt.bfloat16)
            amax_w = stats.tile([P, 1], dt.float32)
            for j in range(n_dma):
                nc.sync.dma_start(wb[:, ts(j, dma_k)], wt_v[:, ts(j, dma_k)])
                chunk2d = wb[:, ts(j, dma_k)].rearrange("p a b -> p (a b)")
                absmax_chunk(chunk2d, amax_w, j == 0, j == n_dma - 1)

            amax_x_b = stats.tile([P, 1], dt.float32)
            nc.gpsimd.partition_all_reduce(
                amax_x_b, amax_x, channels=P, reduce_op=bass_isa.ReduceOp.max
            )

            # ---- global x absmax across cores. AllGather of each core's
            # (uniform) 512B absmax column + a local max over the gathered
            # values -- AllGather has a much lower latency floor than
            # AllReduce (one wire pass, no CCE reduce).
            cc_in = dram.tile([P], dt.float32)
            cc_in_v = cc_in.rearrange("(o p) -> p o", p=P)
            nc.sync.dma_start(cc_in_v, amax_x_b)
            if n_cores > 1:
                cc_out = dram.tile([n_cores * P], dt.float32, addr_space="Shared")
                nc.gpsimd.collective_compute(
                    "AllGather",
                    mybir.AluOpType.bypass,
                    replica_groups=rg,
                    ins=[cc_in.opt()],
                    outs=[cc_out.opt()],
                )
                gath = stats.tile([1, n_cores * P], dt.float32)
                nc.sync.dma_start(gath, cc_out.rearrange("(o f) -> o f", o=1))
            else:
                gath = None

            amax_w_b = stats.tile([P, 1], dt.float32)
            nc.gpsimd.partition_all_reduce(
                amax_w_b, amax_w, channels=P, reduce_op=bass_isa.ReduceOp.max
            )

            def ema_scale(amax_col, prev_col, name):
                t = stats.tile([P, 1], dt.float32, name=f"t_{name}")
                nc.vector.tensor_scalar_add(t, amax_col, 1e-12)
                nc.vector.reciprocal(t, t)
                nc.vector.tensor_scalar_mul(t, t, FP8_MAX)
                nc.vector.tensor_scalar(
                    t, t, 1e-6, 1e6, mybir.AluOpType.max, mybir.AluOpType.min
                )
                s = stats.tile([P, 1], dt.float32, name=f"s_{name}")
                nc.vector.tensor_scalar_mul(s, t, 1.0 - EMA)
                t2 = stats.tile([P, 1], dt.float32, name=f"t2_{name}")
                nc.vector.tensor_scalar_mul(t2, prev_col, EMA)
                nc.vector.tensor_add(s, s, t2)
                return s

            # w-side chain first: nothing here depends on the collective,
            # so DVE stays busy while the AllReduce is in flight.
            s_w = ema_scale(amax_w_b, prev_b[:, 1:2], "w")
            wf = wf_pool.tile([P, KT, N], dt.float8e4)
            n_act = max(1, (3 * n_dma) // 8)  # tail chunks quantized on ACT
            for j in range(n_dma - n_act):
                nc.vector.tensor_scalar_mul(
                    wf[:, ts(j, dma_k)], wb[:, ts(j, dma_k)], s_w
                )
            for j in range(n_dma - n_act, n_dma):
                for t in range(dma_k):
                    nc.scalar.mul(wf[:, j * dma_k + t], wb[:, j * dma_k + t], mul=s_w)

            # x-side chain: gated by the collective result. The local
            # reduction of the gathered absmax values is emitted here, after
            # all w-side DVE work, so the collective wait can't stall it.
            if gath is not None:
                red = stats.tile([1, 1], dt.float32)
                nc.vector.tensor_reduce(
                    red, gath, axis=mybir.AxisListType.X, op=MX
                )
                amax_x_g = stats.tile([P, 1], dt.float32)
                nc.gpsimd.partition_broadcast(amax_x_g, red, channels=P)
            else:
                amax_x_g = amax_x_b
            s_x = ema_scale(amax_x_g, prev_b[:, 0:1], "x")
            xf = xf_pool.tile([P, KT, M], dt.float8e4)
            for j in range(n_dma - n_act):
                nc.vector.tensor_scalar_mul(
                    xf[:, ts(j, dma_k)], xb[:, ts(j, dma_k)], s_x
                )
            for j in range(n_dma - n_act, n_dma):
                for t in range(dma_k):
                    nc.scalar.mul(xf[:, j * dma_k + t], xb[:, j * dma_k + t], mul=s_x)

            # inv = 1 / (s_x * s_w) for the output dequant (emitted after the
            # quantize loops so it doesn't delay the first xf chunk)
            inv = stats.tile([P, 1], dt.float32)
            nc.vector.tensor_mul(inv, s_x, s_w)
            nc.vector.reciprocal(inv, inv)

            # ---- fp8 DoubleRow matmul + dequant epilogue
            for m in range(MT):
                pts = [
                    psum.tile([P, N_TILE], dt.float32, name=f"pt{n}") for n in range(NT)
                ]
                for kk in range(KT // 2):
                    for n in range(NT):
                        nc.tensor.matmul(
                            pts[n],
                            xf[:, 2 * kk : 2 * kk + 2, ts(m, P)],
                            wf[:, 2 * kk : 2 * kk + 2, ts(n, N_TILE)],
                            start=(kk == 0),
                            stop=(kk == KT // 2 - 1),
                            perf_mode=mybir.MatmulPerfMode.DoubleRow,
                        )
                for n in range(NT):
                    out_mn = outp.tile([P, N_TILE], dt.bfloat16, name="out_mn")
                    # split the dequant epilogue across ACT and DVE (DVE is
                    # idle during the dense matmul phase) so PSUM drains and
                    # the final-tile tail are twice as fast
                    if n % 2 == 0:
                        nc.scalar.mul(out_mn, pts[n], mul=inv)
                    else:
                        nc.vector.tensor_scalar_mul(out_mn, pts[n], inv)
                    nc.sync.dma_start(out_v[:, m, ts(n, N_TILE)], out_mn)

    nc.compile()
    return nc


_NC_CACHE = {}


def _get_nc(M, K, N, n_cores=N_CORES):
    key = (M, K, N, n_cores)
    if key not in _NC_CACHE:
        _NC_CACHE[key] = build_nc(M, K, N, n_cores)
    return _NC_CACHE[key]


def run_sharded(x2d, weight, input_scale, weight_scale, n_cores=N_CORES, trace=False):
    """x2d: [rows, K] bf16, weight: [N, K] bf16. Returns ([rows, N] bf16, result)."""
    rows, k = x2d.shape
    n = weight.shape[0]
    m_per = rows // n_cores
    nc = _get_nc(m_per, k, n, n_cores)

    wt = np.ascontiguousarray(weight.T)  # [K, N]
    in_s = np.asarray(input_scale, dtype=np.float32).reshape(1)
    w_s = np.asarray(weight_scale, dtype=np.float32).reshape(1)
    in_maps = []
    for i in range(n_cores):
        xt_i = np.ascontiguousarray(x2d[i * m_per : (i + 1) * m_per].T)  # [K, M]
        in_maps.append({"xt": xt_i, "wt": wt, "in_s": in_s, "w_s": w_s})

    res = run_bass_kernel_spmd(nc, in_maps, core_ids=list(range(n_cores)), trace=trace)
    out = np.concatenate([res.results[i]["out"] for i in range(n_cores)], axis=0)
    return out, res


def kernel(x, weight, input_scale, weight_scale):
    x = np.asarray(x)
    weight = np.asarray(weight)
    b, s, k = x.shape
    x2d = np.ascontiguousarray(x.reshape(b * s, k))
    out, _ = run_sharded(x2d, weight, input_scale, weight_scale)
    return out.reshape(b, s, weight.shape[0]).astype(ml_dtypes.bfloat16)
